# revision 34
# baseline (speedup 1.0000x reference)
"""Banded (sliding-window) causal multi-head attention for Trainium2.

Problem: B=1, H=16, S=2048, DK=64 fp32; layer_idx=1 -> causal mask AND
(i - j) < 256 sliding window.  Returns (context, k, v) like the reference.

Sharding: 16 heads over 8 cores = 2 heads/core (pure head parallelism, no
inter-core communication).

Per-core algorithm (v3.7).  Work is split into tasks, one per (head,
key-block group); the two heads' pipelines are interleaved task-major and
the group sizes are GROUPS (the tail groups are small so the final serial
drain exp+mask+PV of the last task is short).  Per task:
  - QK scores per kb as up-to-three 128-col matmuls (diag / mid / far
    query block) into a [128, 3*nkb*128] PSUM tile laid out
    [d0 f0 d1 f1 ... | m0 m1 ...]; one flat ACT exp per task writes e
    fp16 (ACT exp is ~12.3us/core busy, the second-busiest engine).
  - One DVE multiply over the d/f half with a [128, 256] diag|far 0/1
    mask broadcast via a stride-0 AP produces the masked pt tile.
  - PV accumulates P^T slices against V_aug = [V | ones] (ones column =
    softmax denominator) into a [128, nkb*65] fp32 PSUM tile, mid blocks
    first (their lhsT needs only exp, not the mask, so the PE does not
    stall on the DVE).
  - The raw (unnormalized) ctx+denominator tile is cast PSUM->SBUF fp16
    on the DVE and DMA'd out on the sync ring.  The divide happens on the
    HOST, which removes reciprocal + broadcast-multiply from the device.

Schedule: pairs QK(2p), QK(2p+1), PV(2p-2), PV(2p-1), exp(2p), exp(2p+1)
-- PV lags QK by two tasks so the PE (the busiest engine, ~15us/core at
its sustained mid p-state) never waits on exp/mask, and pairing halves
the ~240ns 64-row<->128-row LDWEIGHTS pipeline drains.

DMA: kt+va0 on the sync ring, qt+va1 on the scalar ring (all scalar-queue
issues happen before the first ACTIVATE so they never displace exp).  The
first chunk of each input gates the first QK and is kept small; the rest
moves as one wide DMA (3KB per-partition descriptors run the rings ~2x
faster than 1KB ones).  DMA-completion semaphores lag the data by ~1us,
so finer chunking buys nothing.  Outputs ride the sync ring.
"""

import os
import sys

for _p in ("/opt/trn_rl_repo", os.path.expanduser("~/.axon_site/_ro/trn_rl_repo")):
    if os.path.isdir(_p) and _p not in sys.path:
        sys.path.insert(0, _p)

import numpy as np

B, H, S, DK = 1, 16, 2048, 64
LOCAL_WINDOW = 256
N_CORES = 8
HPC = H // N_CORES  # heads per core
TB = 128            # tile block
NKB = S // TB       # key blocks per head
G = 4               # key/query blocks per group
NG = NKB // G       # groups per head
VW = DK + 1         # V columns + ones column
GW = 3 * G * TB     # st group tile width: 12 blocks of 128 = 1536
# key-block groups per head: the last group is split small so the serial
# drain (exp+mask+PV of the final task) is short.
GROUPS = [(0, 4), (4, 4), (8, 4), (12, 3), (15, 1)]  # (kb0, nkb)
NT = HPC * len(GROUPS)  # tasks per core

_prog_cache = {}


def _build_banded():
    import concourse.bass as bass
    import concourse.tile as tile
    from concourse import bacc, mybir

    fp16 = mybir.dt.float16
    fp32 = mybir.dt.float32

    nc = bacc.Bacc("TRN2", target_bir_lowering=False, debug=False)
    qt_d = nc.dram_tensor("qt", [TB, S], fp16, kind="ExternalInput")
    kt_d = nc.dram_tensor("kt", [TB, S], fp16, kind="ExternalInput")
    va_d = nc.dram_tensor("va", [TB, HPC * NKB * VW], fp16, kind="ExternalInput")
    ctx_d = nc.dram_tensor("ctx", [TB, HPC * NKB * VW], fp16,
                           kind="ExternalOutput")

    with tile.TileContext(nc) as tc:
        with (
            tc.tile_pool(name="inp", bufs=1) as inp,
            tc.tile_pool(name="exp", bufs=4) as expp,
            tc.tile_pool(name="pt", bufs=4) as ptp,
            tc.tile_pool(name="stp", bufs=2, space="PSUM") as stp,
            tc.tile_pool(name="ctxp", bufs=2, space="PSUM") as ctxp,
            tc.tile_pool(name="outp", bufs=3) as outp,
        ):
            # ---- input tiles ----
            qt_sb = inp.tile([TB, S], fp16, tag="qt")
            kt_sb = inp.tile([TB, S], fp16, tag="kt")
            va_sb = [inp.tile([TB, NKB * VW], fp16, tag=f"va{h}",
                              name=f"va_sb{h}") for h in range(HPC)]
            mask_sb = inp.tile([TB, 2 * TB], fp16, tag="mask")

            # priority-ordered chunks: small first chunks so the first QK
            # matmuls start as early as possible.  All scalar(ACT)-queue
            # issues happen BEFORE the first ACTIVATE, so they don't cost
            # exp throughput; outputs go on the (otherwise idle) sync ring.
            va_cs = NKB * VW
            # full-group chunks in consumption order (DMA-completion
            # semaphores lag the data by ~1us, so per-chunk granularity
            # finer than a group buys nothing): group-0 cols, group-1
            # cols, va (needed by PV(0)/PV(1) around T+4us), then the rest.
            # chunk 1 gates the first QK; the rest moves as ONE wide DMA per
            # tensor (3KB per-partition descriptors run the ring ~2x faster
            # than 1KB ones).  va halves land just ahead of PV(0)/PV(1).
            nc.sync.dma_start(kt_sb[:, 0:512], kt_d.ap()[:, 0:512])
            nc.scalar.dma_start(qt_sb[:, 0:768], qt_d.ap()[:, 0:768])
            nc.sync.dma_start(kt_sb[:, 512:2048], kt_d.ap()[:, 512:2048])
            nc.scalar.dma_start(qt_sb[:, 768:2048], qt_d.ap()[:, 768:2048])
            hvw = va_cs // 2
            nc.sync.dma_start(va_sb[0][:, 0:hvw], va_d.ap()[:, 0:hvw])
            nc.scalar.dma_start(va_sb[1][:, 0:hvw],
                                va_d.ap()[:, va_cs:va_cs + hvw])
            nc.sync.dma_start(va_sb[0][:, hvw:va_cs],
                              va_d.ap()[:, hvw:va_cs])
            nc.scalar.dma_start(va_sb[1][:, hvw:va_cs],
                                va_d.ap()[:, va_cs + hvw:2 * va_cs])

            # ---- on-device band mask: [diag | far] 0/1 patterns ----
            # diag: keep q-offset c >= key-row kl  (causal within block)
            # far:  keep c < kl                    (window edge)
            nc.gpsimd.memset(mask_sb[:], 1.0)
            nc.gpsimd.affine_select(
                mask_sb[:, 0:TB], mask_sb[:, 0:TB],
                pattern=[[1, TB]], compare_op=mybir.AluOpType.is_ge,
                fill=0.0, base=0, channel_multiplier=-1)
            nc.gpsimd.affine_select(
                mask_sb[:, TB:2 * TB], mask_sb[:, TB:2 * TB],
                pattern=[[-1, TB]], compare_op=mybir.AluOpType.is_ge,
                fill=0.0, base=-1, channel_multiplier=1)

            # single-head task order: all of head 0's groups, then head 1's.
            # Consecutive QK tasks then share the same PE row group (h0/h64),
            # avoiding the ~240ns weight-switch pipeline drain per task.
            tasks = [(h, kb0, nkb)
                     for h in range(HPC) for kb0, nkb in GROUPS]

            diag_sl = {}
            far_sl = {}
            mid_sl = {}

            def emit_qk(t):
                h, kb0, nkb = tasks[t]
                hr = slice(h * DK, (h + 1) * DK)
                st = stp.tile([TB, GW], fp32, tag="st", name=f"st_{t}")
                # layout: [d0 f0 d1 f1 ...] then [m0 m1 ...], contiguous.
                # skip blocks whose query block is past the sequence end
                # (their pt/e slices are never consumed by PV).
                specs = []
                for i in range(nkb):
                    kb = kb0 + i
                    for dst, src in [
                        (2 * i * TB, kb * TB),                   # diag
                        ((2 * nkb + i) * TB, kb * TB + TB),      # mid
                        ((2 * i + 1) * TB, kb * TB + 2 * TB),    # far
                    ]:
                        if src + TB <= S:
                            specs.append((dst, src, kb))

                bank_last = {}
                for dst, src, kb in specs:
                    bank_last[dst // 512] = dst
                started_banks = set()
                for dst, src, kb in specs:
                    bank = dst // 512
                    nc.tensor.matmul(
                        st[:, dst:dst + TB],
                        lhsT=kt_sb[hr, kb * TB:(kb + 1) * TB],
                        rhs=qt_sb[hr, src:src + TB],
                        start=(bank not in started_banks),
                        stop=(bank_last[bank] == dst))
                    started_banks.add(bank)
                return st

            def emit_exp_mask(t, st):
                h, kb0, nkb = tasks[t]
                w = 3 * nkb * TB
                e = expp.tile([TB, GW], fp16, tag="exp", name=f"e_{t}")
                nc.scalar.activation(
                    e[:, 0:w], st[:, 0:w], mybir.ActivationFunctionType.Exp)
                pt = ptp.tile([TB, 2 * G * TB], fp16, tag="pt", name=f"pt_{t}")
                dfw = 2 * nkb * TB
                e3 = e[:, 0:dfw].rearrange("p (b c) -> p b c", c=2 * TB)
                p3 = pt[:, 0:dfw].rearrange("p (b c) -> p b c", c=2 * TB)
                m3 = mask_sb[:].unsqueeze(1).broadcast_to([TB, nkb, 2 * TB])
                nc.vector.tensor_mul(p3, e3, m3)
                for i in range(nkb):
                    kb = kb0 + i
                    diag_sl[(h, kb)] = pt[:, 2 * i * TB:(2 * i + 1) * TB]
                    far_sl[(h, kb)] = pt[:, (2 * i + 1) * TB:(2 * i + 2) * TB]
                    mid_sl[(h, kb)] = e[:, (2 * nkb + i) * TB:
                                        (2 * nkb + i + 1) * TB]

            def emit_pv_out(t):
                h, kb0, nkb = tasks[t]
                ct = ctxp.tile([TB, G * VW], fp32, tag="ctx", name=f"ctx_{t}")
                first = True
                for j in range(nkb):
                    qb = kb0 + j
                    # mid first: its lhsT (e slice) only needs exp, while
                    # diag/far need the DVE mask -- by the time the mids
                    # stream, the mask has landed, so the PE never stalls.
                    parts = []
                    if qb >= 1:
                        parts.append((mid_sl[(h, qb - 1)], qb - 1))
                    if qb >= 2:
                        parts.append((far_sl[(h, qb - 2)], qb - 2))
                    parts.append((diag_sl[(h, qb)], qb))
                    for sl, kb in parts:
                        last = (j == nkb - 1) and (kb == qb)
                        nc.tensor.matmul(
                            ct[:, j * VW:(j + 1) * VW], lhsT=sl,
                            rhs=va_sb[h][:, kb * VW:(kb + 1) * VW],
                            start=first, stop=last)
                        first = False
                # raw ctx+denominator: PSUM fp32 -> SBUF fp16 (GpSimd can't
                # read PSUM on TRN2); normalization happens host-side.
                ow = nkb * VW
                o = outp.tile([TB, G * VW], fp16, tag="out", name=f"o_{t}")
                nc.vector.tensor_copy(o[:, 0:ow], ct[:, 0:ow])
                c0 = (h * NKB + kb0) * VW
                nc.sync.dma_start(ctx_d.ap()[:, c0:c0 + ow], o[:, 0:ow])

            # paired schedule: QK(2p), QK(2p+1), PV(2p-2), PV(2p-1) --
            # one QK->PV weight-width switch per pair instead of per task
            # (each 64-row <-> 128-row LDWEIGHTS switch drains the PE
            # pipeline for ~240ns).
            st_tiles = {}
            for p in range(NT // 2 + 1):
                for t in (2 * p, 2 * p + 1):
                    if t < NT:
                        st_tiles[t] = emit_qk(t)
                for t in (2 * p - 2, 2 * p - 1):
                    if 0 <= t:
                        emit_pv_out(t)
                for t in (2 * p, 2 * p + 1):
                    if t < NT:
                        emit_exp_mask(t, st_tiles.pop(t))
    nc.finalize()
    return nc


def _build_causal():
    """Correctness fallback for even layer_idx (full causal attention)."""
    import concourse.bass as bass
    import concourse.tile as tile
    from concourse import bacc, mybir

    fp16 = mybir.dt.float16
    fp32 = mybir.dt.float32
    mwidth = 512

    nc = bacc.Bacc("TRN2", target_bir_lowering=False, debug=False)
    qt_d = nc.dram_tensor("qt", [TB, S], fp16, kind="ExternalInput")
    kt_d = nc.dram_tensor("kt", [TB, S], fp16, kind="ExternalInput")
    va_d = nc.dram_tensor("va", [TB, HPC * NKB * VW], fp16, kind="ExternalInput")
    mask_d = nc.dram_tensor("mask", [TB, mwidth], fp16, kind="ExternalInput")
    ctx_d = nc.dram_tensor("ctx", [HPC, S, DK], fp32, kind="ExternalOutput")

    with tile.TileContext(nc) as tc:
        with (
            tc.tile_pool(name="inp", bufs=1) as inp,
            tc.tile_pool(name="exp", bufs=3) as expp,
            tc.tile_pool(name="pt", bufs=4) as ptp,
            tc.tile_pool(name="stp", bufs=2, space="PSUM") as stp,
            tc.tile_pool(name="ctxp", bufs=4, space="PSUM") as ctxp,
            tc.tile_pool(name="outp", bufs=3) as outp,
        ):
            mask_sb = inp.tile([TB, mwidth], fp16, tag="mask")
            nc.sync.dma_start(mask_sb[:], mask_d.ap())
            qt_sb = inp.tile([TB, S], fp16, tag="qt")
            nc.sync.dma_start(qt_sb[:], qt_d.ap())
            kt_sb = inp.tile([TB, S], fp16, tag="kt")
            nc.sync.dma_start(kt_sb[:], kt_d.ap())
            va_sb = inp.tile([TB, HPC * NKB * VW], fp16, tag="va")
            nc.sync.dma_start(va_sb[:], va_d.ap())

            for h in range(HPC):
                hr = slice(h * DK, (h + 1) * DK)
                ctx_tiles = {}
                started = set()
                for kb in range(NKB):
                    span = S - kb * TB
                    chunks = []
                    for o in range(0, span, 512):
                        w = min(512, span - o)
                        st = stp.tile([TB, 512], fp32, tag="st",
                                      name=f"st_{h}_{kb}_{o}")
                        nc.tensor.matmul(
                            st[:, 0:w], lhsT=kt_sb[hr, kb * TB:kb * TB + TB],
                            rhs=qt_sb[hr, kb * TB + o:kb * TB + o + w],
                            start=True, stop=True)
                        pt = ptp.tile([TB, 512], fp16, tag="pt",
                                      name=f"pt_{h}_{kb}_{o}")
                        if o == 0:
                            e = expp.tile([TB, 512], fp16, tag="exp",
                                          name=f"e_{h}_{kb}_{o}")
                            nc.scalar.activation(
                                e[:, 0:w], st[:, 0:w],
                                mybir.ActivationFunctionType.Exp)
                            nc.vector.tensor_mul(
                                pt[:, 0:w], e[:, 0:w], mask_sb[:, 0:w])
                        else:
                            nc.scalar.activation(
                                pt[:, 0:w], st[:, 0:w],
                                mybir.ActivationFunctionType.Exp)
                        chunks.append(pt)

                    for qb in range(kb, NKB):
                        g, j = divmod(qb, G)
                        if g not in ctx_tiles:
                            ctx_tiles[g] = ctxp.tile(
                                [TB, G * VW], fp32, tag="ctx", name=f"ctx_{h}_{g}")
                        ct = ctx_tiles[g]
                        o = (qb - kb) * TB
                        src = chunks[o // 512]
                        oo = o % 512
                        last = (qb == g * G + G - 1) and (kb == qb)
                        nc.tensor.matmul(
                            ct[:, j * VW:(j + 1) * VW],
                            lhsT=src[:, oo:oo + TB],
                            rhs=va_sb[:, (h * NKB + kb) * VW:(h * NKB + kb + 1) * VW],
                            start=(g not in started), stop=last)
                        started.add(g)
                        if last:
                            ct3 = ct[:].rearrange("p (n c) -> p n c", c=VW)
                            recip = outp.tile([TB, G], fp32, tag="recip",
                                              name=f"recip_{h}_{g}")
                            nc.vector.reciprocal(recip[:], ct3[:, :, DK])
                            out_sb = outp.tile([TB, G * DK], fp32, tag="out",
                                               name=f"out_{h}_{g}")
                            out3 = out_sb[:].rearrange("p (n c) -> p n c", c=DK)
                            nc.vector.tensor_mul(
                                out3, ct3[:, :, 0:DK],
                                recip[:].unsqueeze(2).broadcast_to([TB, G, DK]))
                            dst = ctx_d.ap()[h, g * G * TB:(g + 1) * G * TB, :]
                            dst = dst.rearrange("(n p) d -> p n d", p=TB)
                            nc.sync.dma_start(dst, out3)
                            del ctx_tiles[g]
                            started.discard(g)
    nc.finalize()
    return nc


def _get_program(win):
    if win not in _prog_cache:
        _prog_cache[win] = (
            _build_banded() if win == LOCAL_WINDOW else _build_causal())
    return _prog_cache[win]


def _make_mask_np_causal():
    kl = np.arange(TB)[:, None]
    qs = np.arange(512)[None, :]
    return ((qs - kl) >= 0).astype(np.float16)


def make_in_maps(q, k, v, win):
    scale = np.float32(1.0 / np.sqrt(DK))
    in_maps = []
    for c in range(N_CORES):
        heads = range(c * HPC, (c + 1) * HPC)
        qt = np.concatenate(
            [(q[0, h] * scale).T for h in heads], axis=0).astype(np.float16)
        kt = np.concatenate(
            [k[0, h].T for h in heads], axis=0).astype(np.float16)
        va = np.empty((TB, HPC * NKB * VW), np.float16)
        for hi, h in enumerate(heads):
            vh = np.concatenate(
                [v[0, h], np.ones((S, 1), np.float32)], axis=1)  # [S, 65]
            va[:, hi * NKB * VW:(hi + 1) * NKB * VW] = (
                vh.reshape(NKB, TB, VW).transpose(1, 0, 2).reshape(TB, NKB * VW)
            ).astype(np.float16)
        m = {
            "qt": np.ascontiguousarray(qt),
            "kt": np.ascontiguousarray(kt),
            "va": np.ascontiguousarray(va),
        }
        if win != LOCAL_WINDOW:
            m["mask"] = _make_mask_np_causal()
        in_maps.append(m)
    return in_maps


def decode_ctx(out, win):
    """Decode one core's 'ctx' result to [HPC, S, DK] fp32."""
    if win != LOCAL_WINDOW:
        return np.asarray(out, np.float32)
    # banded layout: [TB, HPC*NKB*VW] fp16 raw ctx+denominator; column
    # (h*NKB + qb)*VW + c holds ctx (c<DK) / denom (c=DK) for query
    # qb*TB + p of head h.
    a = np.asarray(out, np.float32).reshape(TB, HPC, NKB, VW)
    o = a[..., 0:DK] / a[..., DK:DK + 1]        # [TB, HPC, NKB, DK]
    o = o.transpose(1, 2, 0, 3)                 # [HPC, NKB, TB, DK]
    return np.ascontiguousarray(o.reshape(HPC, S, DK))


def kernel(q, k, v, layer_idx=1, training=0):
    from concourse.bass_utils import run_bass_kernel_spmd

    q = np.asarray(q)
    k = np.asarray(k)
    v = np.asarray(v)
    li = int(np.asarray(layer_idx))
    win = S if li % 2 == 0 else LOCAL_WINDOW

    nc = _get_program(win)
    in_maps = make_in_maps(q, k, v, win)
    res = run_bass_kernel_spmd(nc, in_maps, core_ids=list(range(N_CORES)))

    ctx = np.empty((B, H, S, DK), np.float32)
    for c in range(N_CORES):
        out = decode_ctx(res.results[c]["ctx"], win)
        for hi in range(HPC):
            ctx[0, c * HPC + hi] = out[hi]
    return ctx, k, v


# revision 39
# speedup vs baseline: 1.1146x; 1.1146x over previous
"""Banded (sliding-window) causal multi-head attention for Trainium2.

Problem: B=1, H=16, S=2048, DK=64 fp32; layer_idx=1 -> causal mask AND
(i - j) < 256 sliding window.  Returns (context, k, v) like the reference.

Sharding: 16 heads over 8 cores = 2 heads/core (pure head parallelism, no
inter-core communication).

Per-core algorithm (v3.7).  Work is split into tasks, one per (head,
key-block group); the two heads' pipelines are interleaved task-major and
the group sizes are GROUPS (the tail groups are small so the final serial
drain exp+mask+PV of the last task is short).  Per task:
  - QK scores per kb as up-to-three 128-col matmuls (diag / mid / far
    query block) into a [128, 3*nkb*128] PSUM tile laid out
    [d0 f0 d1 f1 ... | m0 m1 ...]; one flat ACT exp per task writes e
    fp16 (ACT exp is ~12.3us/core busy, the second-busiest engine).
  - One DVE multiply over the d/f half with a [128, 256] diag|far 0/1
    mask broadcast via a stride-0 AP produces the masked pt tile.
  - PV accumulates P^T slices against V_aug = [V | ones] (ones column =
    softmax denominator) into a [128, nkb*65] fp32 PSUM tile, mid blocks
    first (their lhsT needs only exp, not the mask, so the PE does not
    stall on the DVE).
  - The raw (unnormalized) ctx+denominator tile is cast PSUM->SBUF fp16
    on the DVE and DMA'd out on the sync ring.  The divide happens on the
    HOST, which removes reciprocal + broadcast-multiply from the device.

Schedule: pairs QK(2p), QK(2p+1), PV(2p-2), PV(2p-1), exp(2p), exp(2p+1)
-- PV lags QK by two tasks so the PE (the busiest engine, ~15us/core at
its sustained mid p-state) never waits on exp/mask, and pairing halves
the ~240ns 64-row<->128-row LDWEIGHTS pipeline drains.

DMA: kt+va0 on the sync ring, qt+va1 on the scalar ring (all scalar-queue
issues happen before the first ACTIVATE so they never displace exp).  The
first chunk of each input gates the first QK and is kept small; the rest
moves as one wide DMA (3KB per-partition descriptors run the rings ~2x
faster than 1KB ones).  DMA-completion semaphores lag the data by ~1us,
so finer chunking buys nothing.  Outputs ride the sync ring.
"""

import os
import sys

for _p in ("/opt/trn_rl_repo", os.path.expanduser("~/.axon_site/_ro/trn_rl_repo")):
    if os.path.isdir(_p) and _p not in sys.path:
        sys.path.insert(0, _p)

import numpy as np

B, H, S, DK = 1, 16, 2048, 64
LOCAL_WINDOW = 256
N_CORES = 8
HPC = H // N_CORES  # heads per core
TB = 128            # tile block
NKB = S // TB       # key blocks per head
G = 4               # key/query blocks per group
NG = NKB // G       # groups per head
VW = DK + 1         # V columns + ones column
GW = 3 * G * TB     # st group tile width: 12 blocks of 128 = 1536
# key-block groups per head: the last group is split small so the serial
# drain (exp+mask+PV of the final task) is short.
GROUPS = [(0, 4), (4, 4), (8, 4), (12, 3), (15, 1)]  # (kb0, nkb)
NT = HPC * len(GROUPS)  # tasks per core

_prog_cache = {}


def _build_banded():
    import concourse.bass as bass
    import concourse.tile as tile
    from concourse import bacc, mybir

    fp16 = mybir.dt.float16
    fp32 = mybir.dt.float32

    nc = bacc.Bacc("TRN2", target_bir_lowering=False, debug=False)
    qt_d = nc.dram_tensor("qt", [TB, S], fp16, kind="ExternalInput")
    kt_d = nc.dram_tensor("kt", [TB, S], fp16, kind="ExternalInput")
    va_d = nc.dram_tensor("va", [TB, HPC * NKB * VW], fp16, kind="ExternalInput")
    ctx_d = nc.dram_tensor("ctx", [TB, HPC * NKB * VW], fp16,
                           kind="ExternalOutput")

    with tile.TileContext(nc) as tc:
        with (
            tc.tile_pool(name="inp", bufs=1) as inp,
            tc.tile_pool(name="exp", bufs=4) as expp,
            tc.tile_pool(name="pt", bufs=4) as ptp,
            tc.tile_pool(name="stp", bufs=2, space="PSUM") as stp,
            tc.tile_pool(name="ctxp", bufs=2, space="PSUM") as ctxp,
            tc.tile_pool(name="outp", bufs=3) as outp,
        ):
            # ---- input tiles ----
            # qt is stored per head, zero-padded in the other head's rows:
            # QK then uses the FULL two-head kt tile as a 128-row lhsT (the
            # zero qt rows null the other head's contribution), so every
            # LDWEIGHTS in the kernel is 128 rows wide and the ~240ns
            # 64<->128-row weight-width pipeline drains disappear.
            qt_sb = [inp.tile([TB, S], fp16, tag=f"qt{h}", name=f"qt_sb{h}")
                     for h in range(HPC)]
            kt_sb = inp.tile([TB, S], fp16, tag="kt")
            nc.vector.memset(qt_sb[0][DK:TB, :], 0.0)
            nc.vector.memset(qt_sb[1][0:DK, :], 0.0)
            va_sb = [inp.tile([TB, NKB * VW], fp16, tag=f"va{h}",
                              name=f"va_sb{h}") for h in range(HPC)]
            mask_sb = inp.tile([TB, 2 * TB], fp16, tag="mask")

            # priority-ordered chunks: small first chunks so the first QK
            # matmuls start as early as possible.  All scalar(ACT)-queue
            # issues happen BEFORE the first ACTIVATE, so they don't cost
            # exp throughput; outputs go on the (otherwise idle) sync ring.
            va_cs = NKB * VW
            # full-group chunks in consumption order (DMA-completion
            # semaphores lag the data by ~1us, so per-chunk granularity
            # finer than a group buys nothing): group-0 cols, group-1
            # cols, va (needed by PV(0)/PV(1) around T+4us), then the rest.
            # chunk 1 gates the first QK; the rest moves as ONE wide DMA per
            # tensor (3KB per-partition descriptors run the ring ~2x faster
            # than 1KB ones).  va halves land just ahead of PV(0)/PV(1).
            nc.sync.dma_start(kt_sb[:, 0:512], kt_d.ap()[:, 0:512])
            nc.scalar.dma_start(qt_sb[0][0:DK, 0:768], qt_d.ap()[0:DK, 0:768])
            nc.scalar.dma_start(qt_sb[1][DK:TB, 0:768], qt_d.ap()[DK:TB, 0:768])
            nc.sync.dma_start(kt_sb[:, 512:2048], kt_d.ap()[:, 512:2048])
            nc.scalar.dma_start(qt_sb[0][0:DK, 768:2048],
                                qt_d.ap()[0:DK, 768:2048])
            nc.scalar.dma_start(qt_sb[1][DK:TB, 768:2048],
                                qt_d.ap()[DK:TB, 768:2048])
            hvw = va_cs // 2
            nc.sync.dma_start(va_sb[0][:, 0:hvw], va_d.ap()[:, 0:hvw])
            nc.scalar.dma_start(va_sb[1][:, 0:hvw],
                                va_d.ap()[:, va_cs:va_cs + hvw])
            nc.sync.dma_start(va_sb[0][:, hvw:va_cs],
                              va_d.ap()[:, hvw:va_cs])
            nc.scalar.dma_start(va_sb[1][:, hvw:va_cs],
                                va_d.ap()[:, va_cs + hvw:2 * va_cs])

            # ---- on-device band mask: [diag | far] 0/1 patterns ----
            # diag: keep q-offset c >= key-row kl  (causal within block)
            # far:  keep c < kl                    (window edge)
            nc.gpsimd.memset(mask_sb[:], 1.0)
            nc.gpsimd.affine_select(
                mask_sb[:, 0:TB], mask_sb[:, 0:TB],
                pattern=[[1, TB]], compare_op=mybir.AluOpType.is_ge,
                fill=0.0, base=0, channel_multiplier=-1)
            nc.gpsimd.affine_select(
                mask_sb[:, TB:2 * TB], mask_sb[:, TB:2 * TB],
                pattern=[[-1, TB]], compare_op=mybir.AluOpType.is_ge,
                fill=0.0, base=-1, channel_multiplier=1)

            # task t = gi*HPC + h -> (head h, group GROUPS[gi])
            tasks = [(h, kb0, nkb)
                     for kb0, nkb in GROUPS for h in range(HPC)]

            diag_sl = {}
            far_sl = {}
            mid_sl = {}

            def emit_qk(t):
                h, kb0, nkb = tasks[t]
                st = stp.tile([TB, GW], fp32, tag="st", name=f"st_{t}")
                # layout: [d0 f0 d1 f1 ...] then [m0 m1 ...], contiguous.
                # skip blocks whose query block is past the sequence end
                # (their pt/e slices are never consumed by PV).
                specs = []
                for i in range(nkb):
                    kb = kb0 + i
                    for dst, src in [
                        (2 * i * TB, kb * TB),                   # diag
                        ((2 * nkb + i) * TB, kb * TB + TB),      # mid
                        ((2 * i + 1) * TB, kb * TB + 2 * TB),    # far
                    ]:
                        if src + TB <= S:
                            specs.append((dst, src, kb))

                bank_last = {}
                for dst, src, kb in specs:
                    bank_last[dst // 512] = dst
                started_banks = set()
                for dst, src, kb in specs:
                    bank = dst // 512
                    nc.tensor.matmul(
                        st[:, dst:dst + TB],
                        lhsT=kt_sb[:, kb * TB:(kb + 1) * TB],
                        rhs=qt_sb[h][:, src:src + TB],
                        start=(bank not in started_banks),
                        stop=(bank_last[bank] == dst))
                    started_banks.add(bank)
                return st

            def emit_exp_mask(t, st):
                h, kb0, nkb = tasks[t]
                w = 3 * nkb * TB
                e = expp.tile([TB, GW], fp16, tag="exp", name=f"e_{t}")
                nc.scalar.activation(
                    e[:, 0:w], st[:, 0:w], mybir.ActivationFunctionType.Exp)
                pt = ptp.tile([TB, 2 * G * TB], fp16, tag="pt", name=f"pt_{t}")
                dfw = 2 * nkb * TB
                e3 = e[:, 0:dfw].rearrange("p (b c) -> p b c", c=2 * TB)
                p3 = pt[:, 0:dfw].rearrange("p (b c) -> p b c", c=2 * TB)
                m3 = mask_sb[:].unsqueeze(1).broadcast_to([TB, nkb, 2 * TB])
                nc.vector.tensor_mul(p3, e3, m3)
                for i in range(nkb):
                    kb = kb0 + i
                    diag_sl[(h, kb)] = pt[:, 2 * i * TB:(2 * i + 1) * TB]
                    far_sl[(h, kb)] = pt[:, (2 * i + 1) * TB:(2 * i + 2) * TB]
                    mid_sl[(h, kb)] = e[:, (2 * nkb + i) * TB:
                                        (2 * nkb + i + 1) * TB]

            def emit_pv_out(t):
                h, kb0, nkb = tasks[t]
                ct = ctxp.tile([TB, G * VW], fp32, tag="ctx", name=f"ctx_{t}")
                first = True
                for j in range(nkb):
                    qb = kb0 + j
                    # mid first: its lhsT (e slice) only needs exp, while
                    # diag/far need the DVE mask -- by the time the mids
                    # stream, the mask has landed, so the PE never stalls.
                    parts = []
                    if qb >= 1:
                        parts.append((mid_sl[(h, qb - 1)], qb - 1))
                    if qb >= 2:
                        parts.append((far_sl[(h, qb - 2)], qb - 2))
                    parts.append((diag_sl[(h, qb)], qb))
                    for sl, kb in parts:
                        last = (j == nkb - 1) and (kb == qb)
                        nc.tensor.matmul(
                            ct[:, j * VW:(j + 1) * VW], lhsT=sl,
                            rhs=va_sb[h][:, kb * VW:(kb + 1) * VW],
                            start=first, stop=last)
                        first = False
                # raw ctx+denominator: PSUM fp32 -> SBUF fp16 (GpSimd can't
                # read PSUM on TRN2); normalization happens host-side.
                ow = nkb * VW
                o = outp.tile([TB, G * VW], fp16, tag="out", name=f"o_{t}")
                nc.vector.tensor_copy(o[:, 0:ow], ct[:, 0:ow])
                c0 = (h * NKB + kb0) * VW
                nc.sync.dma_start(ctx_d.ap()[:, c0:c0 + ow], o[:, 0:ow])

            # paired schedule: QK(2p), QK(2p+1), PV(2p-2), PV(2p-1) --
            # one QK->PV weight-width switch per pair instead of per task
            # (each 64-row <-> 128-row LDWEIGHTS switch drains the PE
            # pipeline for ~240ns).
            st_tiles = {}
            for p in range(NT // 2 + 1):
                for t in (2 * p, 2 * p + 1):
                    if t < NT:
                        st_tiles[t] = emit_qk(t)
                for t in (2 * p - 2, 2 * p - 1):
                    if 0 <= t:
                        emit_pv_out(t)
                for t in (2 * p, 2 * p + 1):
                    if t < NT:
                        emit_exp_mask(t, st_tiles.pop(t))
    nc.finalize()
    return nc


def _build_causal():
    """Correctness fallback for even layer_idx (full causal attention)."""
    import concourse.bass as bass
    import concourse.tile as tile
    from concourse import bacc, mybir

    fp16 = mybir.dt.float16
    fp32 = mybir.dt.float32
    mwidth = 512

    nc = bacc.Bacc("TRN2", target_bir_lowering=False, debug=False)
    qt_d = nc.dram_tensor("qt", [TB, S], fp16, kind="ExternalInput")
    kt_d = nc.dram_tensor("kt", [TB, S], fp16, kind="ExternalInput")
    va_d = nc.dram_tensor("va", [TB, HPC * NKB * VW], fp16, kind="ExternalInput")
    mask_d = nc.dram_tensor("mask", [TB, mwidth], fp16, kind="ExternalInput")
    ctx_d = nc.dram_tensor("ctx", [HPC, S, DK], fp32, kind="ExternalOutput")

    with tile.TileContext(nc) as tc:
        with (
            tc.tile_pool(name="inp", bufs=1) as inp,
            tc.tile_pool(name="exp", bufs=3) as expp,
            tc.tile_pool(name="pt", bufs=4) as ptp,
            tc.tile_pool(name="stp", bufs=2, space="PSUM") as stp,
            tc.tile_pool(name="ctxp", bufs=4, space="PSUM") as ctxp,
            tc.tile_pool(name="outp", bufs=3) as outp,
        ):
            mask_sb = inp.tile([TB, mwidth], fp16, tag="mask")
            nc.sync.dma_start(mask_sb[:], mask_d.ap())
            qt_sb = inp.tile([TB, S], fp16, tag="qt")
            nc.sync.dma_start(qt_sb[:], qt_d.ap())
            kt_sb = inp.tile([TB, S], fp16, tag="kt")
            nc.sync.dma_start(kt_sb[:], kt_d.ap())
            va_sb = inp.tile([TB, HPC * NKB * VW], fp16, tag="va")
            nc.sync.dma_start(va_sb[:], va_d.ap())

            for h in range(HPC):
                hr = slice(h * DK, (h + 1) * DK)
                ctx_tiles = {}
                started = set()
                for kb in range(NKB):
                    span = S - kb * TB
                    chunks = []
                    for o in range(0, span, 512):
                        w = min(512, span - o)
                        st = stp.tile([TB, 512], fp32, tag="st",
                                      name=f"st_{h}_{kb}_{o}")
                        nc.tensor.matmul(
                            st[:, 0:w], lhsT=kt_sb[hr, kb * TB:kb * TB + TB],
                            rhs=qt_sb[hr, kb * TB + o:kb * TB + o + w],
                            start=True, stop=True)
                        pt = ptp.tile([TB, 512], fp16, tag="pt",
                                      name=f"pt_{h}_{kb}_{o}")
                        if o == 0:
                            e = expp.tile([TB, 512], fp16, tag="exp",
                                          name=f"e_{h}_{kb}_{o}")
                            nc.scalar.activation(
                                e[:, 0:w], st[:, 0:w],
                                mybir.ActivationFunctionType.Exp)
                            nc.vector.tensor_mul(
                                pt[:, 0:w], e[:, 0:w], mask_sb[:, 0:w])
                        else:
                            nc.scalar.activation(
                                pt[:, 0:w], st[:, 0:w],
                                mybir.ActivationFunctionType.Exp)
                        chunks.append(pt)

                    for qb in range(kb, NKB):
                        g, j = divmod(qb, G)
                        if g not in ctx_tiles:
                            ctx_tiles[g] = ctxp.tile(
                                [TB, G * VW], fp32, tag="ctx", name=f"ctx_{h}_{g}")
                        ct = ctx_tiles[g]
                        o = (qb - kb) * TB
                        src = chunks[o // 512]
                        oo = o % 512
                        last = (qb == g * G + G - 1) and (kb == qb)
                        nc.tensor.matmul(
                            ct[:, j * VW:(j + 1) * VW],
                            lhsT=src[:, oo:oo + TB],
                            rhs=va_sb[:, (h * NKB + kb) * VW:(h * NKB + kb + 1) * VW],
                            start=(g not in started), stop=last)
                        started.add(g)
                        if last:
                            ct3 = ct[:].rearrange("p (n c) -> p n c", c=VW)
                            recip = outp.tile([TB, G], fp32, tag="recip",
                                              name=f"recip_{h}_{g}")
                            nc.vector.reciprocal(recip[:], ct3[:, :, DK])
                            out_sb = outp.tile([TB, G * DK], fp32, tag="out",
                                               name=f"out_{h}_{g}")
                            out3 = out_sb[:].rearrange("p (n c) -> p n c", c=DK)
                            nc.vector.tensor_mul(
                                out3, ct3[:, :, 0:DK],
                                recip[:].unsqueeze(2).broadcast_to([TB, G, DK]))
                            dst = ctx_d.ap()[h, g * G * TB:(g + 1) * G * TB, :]
                            dst = dst.rearrange("(n p) d -> p n d", p=TB)
                            nc.sync.dma_start(dst, out3)
                            del ctx_tiles[g]
                            started.discard(g)
    nc.finalize()
    return nc


def _get_program(win):
    if win not in _prog_cache:
        _prog_cache[win] = (
            _build_banded() if win == LOCAL_WINDOW else _build_causal())
    return _prog_cache[win]


def _make_mask_np_causal():
    kl = np.arange(TB)[:, None]
    qs = np.arange(512)[None, :]
    return ((qs - kl) >= 0).astype(np.float16)


def make_in_maps(q, k, v, win):
    scale = np.float32(1.0 / np.sqrt(DK))
    in_maps = []
    for c in range(N_CORES):
        heads = range(c * HPC, (c + 1) * HPC)
        qt = np.concatenate(
            [(q[0, h] * scale).T for h in heads], axis=0).astype(np.float16)
        kt = np.concatenate(
            [k[0, h].T for h in heads], axis=0).astype(np.float16)
        va = np.empty((TB, HPC * NKB * VW), np.float16)
        for hi, h in enumerate(heads):
            vh = np.concatenate(
                [v[0, h], np.ones((S, 1), np.float32)], axis=1)  # [S, 65]
            va[:, hi * NKB * VW:(hi + 1) * NKB * VW] = (
                vh.reshape(NKB, TB, VW).transpose(1, 0, 2).reshape(TB, NKB * VW)
            ).astype(np.float16)
        m = {
            "qt": np.ascontiguousarray(qt),
            "kt": np.ascontiguousarray(kt),
            "va": np.ascontiguousarray(va),
        }
        if win != LOCAL_WINDOW:
            m["mask"] = _make_mask_np_causal()
        in_maps.append(m)
    return in_maps


def decode_ctx(out, win):
    """Decode one core's 'ctx' result to [HPC, S, DK] fp32."""
    if win != LOCAL_WINDOW:
        return np.asarray(out, np.float32)
    # banded layout: [TB, HPC*NKB*VW] fp16 raw ctx+denominator; column
    # (h*NKB + qb)*VW + c holds ctx (c<DK) / denom (c=DK) for query
    # qb*TB + p of head h.
    a = np.asarray(out, np.float32).reshape(TB, HPC, NKB, VW)
    o = a[..., 0:DK] / a[..., DK:DK + 1]        # [TB, HPC, NKB, DK]
    o = o.transpose(1, 2, 0, 3)                 # [HPC, NKB, TB, DK]
    return np.ascontiguousarray(o.reshape(HPC, S, DK))


def kernel(q, k, v, layer_idx=1, training=0):
    from concourse.bass_utils import run_bass_kernel_spmd

    q = np.asarray(q)
    k = np.asarray(k)
    v = np.asarray(v)
    li = int(np.asarray(layer_idx))
    win = S if li % 2 == 0 else LOCAL_WINDOW

    nc = _get_program(win)
    in_maps = make_in_maps(q, k, v, win)
    res = run_bass_kernel_spmd(nc, in_maps, core_ids=list(range(N_CORES)))

    ctx = np.empty((B, H, S, DK), np.float32)
    for c in range(N_CORES):
        out = decode_ctx(res.results[c]["ctx"], win)
        for hi in range(HPC):
            ctx[0, c * HPC + hi] = out[hi]
    return ctx, k, v


# revision 40
# speedup vs baseline: 1.1498x; 1.0316x over previous
"""Banded (sliding-window) causal multi-head attention for Trainium2.

Problem: B=1, H=16, S=2048, DK=64 fp32; layer_idx=1 -> causal mask AND
(i - j) < 256 sliding window.  Returns (context, k, v) like the reference.

Sharding: 16 heads over 8 cores = 2 heads/core (pure head parallelism, no
inter-core communication).

Per-core algorithm (v3.7).  Work is split into tasks, one per (head,
key-block group); the two heads' pipelines are interleaved task-major and
the group sizes are GROUPS (the tail groups are small so the final serial
drain exp+mask+PV of the last task is short).  Per task:
  - QK scores per kb as up-to-three 128-col matmuls (diag / mid / far
    query block) into a [128, 3*nkb*128] PSUM tile laid out
    [d0 f0 d1 f1 ... | m0 m1 ...]; one flat ACT exp per task writes e
    fp16 (ACT exp is ~12.3us/core busy, the second-busiest engine).
  - One DVE multiply over the d/f half with a [128, 256] diag|far 0/1
    mask broadcast via a stride-0 AP produces the masked pt tile.
  - PV accumulates P^T slices against V_aug = [V | ones] (ones column =
    softmax denominator) into a [128, nkb*65] fp32 PSUM tile, mid blocks
    first (their lhsT needs only exp, not the mask, so the PE does not
    stall on the DVE).
  - The raw (unnormalized) ctx+denominator tile is cast PSUM->SBUF fp16
    on the DVE and DMA'd out on the sync ring.  The divide happens on the
    HOST, which removes reciprocal + broadcast-multiply from the device.

Schedule: pairs QK(2p), QK(2p+1), PV(2p-2), PV(2p-1), exp(2p), exp(2p+1)
-- PV lags QK by two tasks so the PE (the busiest engine, ~15us/core at
its sustained mid p-state) never waits on exp/mask, and pairing halves
the ~240ns 64-row<->128-row LDWEIGHTS pipeline drains.

DMA: kt+va0 on the sync ring, qt+va1 on the scalar ring (all scalar-queue
issues happen before the first ACTIVATE so they never displace exp).  The
first chunk of each input gates the first QK and is kept small; the rest
moves as one wide DMA (3KB per-partition descriptors run the rings ~2x
faster than 1KB ones).  DMA-completion semaphores lag the data by ~1us,
so finer chunking buys nothing.  Outputs ride the sync ring.
"""

import os
import sys

for _p in ("/opt/trn_rl_repo", os.path.expanduser("~/.axon_site/_ro/trn_rl_repo")):
    if os.path.isdir(_p) and _p not in sys.path:
        sys.path.insert(0, _p)

import numpy as np

B, H, S, DK = 1, 16, 2048, 64
LOCAL_WINDOW = 256
N_CORES = 8
HPC = H // N_CORES  # heads per core
TB = 128            # tile block
NKB = S // TB       # key blocks per head
G = 4               # key/query blocks per group
NG = NKB // G       # groups per head
VW = DK + 1         # V columns + ones column
GW = 3 * G * TB     # st group tile width: 12 blocks of 128 = 1536
# key-block groups per head: the last group is split small so the serial
# drain (exp+mask+PV of the final task) is short.
GROUPS = [(0, 4), (4, 4), (8, 4), (12, 3), (15, 1)]  # (kb0, nkb)
NT = HPC * len(GROUPS)  # tasks per core

_prog_cache = {}


def _build_banded():
    import concourse.bass as bass
    import concourse.tile as tile
    from concourse import bacc, mybir

    fp16 = mybir.dt.float16
    fp32 = mybir.dt.float32

    nc = bacc.Bacc("TRN2", target_bir_lowering=False, debug=False)
    qt_d = nc.dram_tensor("qt", [TB, S], fp16, kind="ExternalInput")
    kt_d = nc.dram_tensor("kt", [TB, S], fp16, kind="ExternalInput")
    va_d = nc.dram_tensor("va", [TB, HPC * NKB * VW], fp16, kind="ExternalInput")
    ctx_d = nc.dram_tensor("ctx", [TB, HPC * NKB * VW], fp16,
                           kind="ExternalOutput")

    with tile.TileContext(nc) as tc:
        with (
            tc.tile_pool(name="inp", bufs=1) as inp,
            tc.tile_pool(name="exp", bufs=4) as expp,
            tc.tile_pool(name="pt", bufs=4) as ptp,
            tc.tile_pool(name="stp", bufs=2, space="PSUM") as stp,
            tc.tile_pool(name="ctxp", bufs=2, space="PSUM") as ctxp,
            tc.tile_pool(name="outp", bufs=8) as outp,
        ):
            # ---- input tiles ----
            # qt is stored per head, zero-padded in the other head's rows:
            # QK then uses the FULL two-head kt tile as a 128-row lhsT (the
            # zero qt rows null the other head's contribution), so every
            # LDWEIGHTS in the kernel is 128 rows wide and the ~240ns
            # 64<->128-row weight-width pipeline drains disappear.
            qt_sb = [inp.tile([TB, S], fp16, tag=f"qt{h}", name=f"qt_sb{h}")
                     for h in range(HPC)]
            kt_sb = inp.tile([TB, S], fp16, tag="kt")
            nc.vector.memset(qt_sb[0][DK:TB, :], 0.0)
            nc.vector.memset(qt_sb[1][0:DK, :], 0.0)
            va_sb = [inp.tile([TB, NKB * VW], fp16, tag=f"va{h}",
                              name=f"va_sb{h}") for h in range(HPC)]
            mask_sb = inp.tile([TB, 2 * TB], fp16, tag="mask")

            # priority-ordered chunks: small first chunks so the first QK
            # matmuls start as early as possible.  All scalar(ACT)-queue
            # issues happen BEFORE the first ACTIVATE, so they don't cost
            # exp throughput; outputs go on the (otherwise idle) sync ring.
            va_cs = NKB * VW
            # full-group chunks in consumption order (DMA-completion
            # semaphores lag the data by ~1us, so per-chunk granularity
            # finer than a group buys nothing): group-0 cols, group-1
            # cols, va (needed by PV(0)/PV(1) around T+4us), then the rest.
            # chunk 1 gates the first QK; the rest moves as ONE wide DMA per
            # tensor (3KB per-partition descriptors run the ring ~2x faster
            # than 1KB ones).  va halves land just ahead of PV(0)/PV(1).
            nc.sync.dma_start(kt_sb[:, 0:512], kt_d.ap()[:, 0:512])
            nc.scalar.dma_start(qt_sb[0][0:DK, 0:768], qt_d.ap()[0:DK, 0:768])
            nc.scalar.dma_start(qt_sb[1][DK:TB, 0:768], qt_d.ap()[DK:TB, 0:768])
            nc.sync.dma_start(kt_sb[:, 512:2048], kt_d.ap()[:, 512:2048])
            nc.scalar.dma_start(qt_sb[0][0:DK, 768:2048],
                                qt_d.ap()[0:DK, 768:2048])
            nc.scalar.dma_start(qt_sb[1][DK:TB, 768:2048],
                                qt_d.ap()[DK:TB, 768:2048])
            hvw = va_cs // 2
            nc.sync.dma_start(va_sb[0][:, 0:hvw], va_d.ap()[:, 0:hvw])
            nc.scalar.dma_start(va_sb[1][:, 0:hvw],
                                va_d.ap()[:, va_cs:va_cs + hvw])
            nc.sync.dma_start(va_sb[0][:, hvw:va_cs],
                              va_d.ap()[:, hvw:va_cs])
            nc.scalar.dma_start(va_sb[1][:, hvw:va_cs],
                                va_d.ap()[:, va_cs + hvw:2 * va_cs])

            # ---- on-device band mask: [diag | far] 0/1 patterns ----
            # diag: keep q-offset c >= key-row kl  (causal within block)
            # far:  keep c < kl                    (window edge)
            nc.gpsimd.memset(mask_sb[:], 1.0)
            nc.gpsimd.affine_select(
                mask_sb[:, 0:TB], mask_sb[:, 0:TB],
                pattern=[[1, TB]], compare_op=mybir.AluOpType.is_ge,
                fill=0.0, base=0, channel_multiplier=-1)
            nc.gpsimd.affine_select(
                mask_sb[:, TB:2 * TB], mask_sb[:, TB:2 * TB],
                pattern=[[-1, TB]], compare_op=mybir.AluOpType.is_ge,
                fill=0.0, base=-1, channel_multiplier=1)

            # task t = gi*HPC + h -> (head h, group GROUPS[gi])
            tasks = [(h, kb0, nkb)
                     for kb0, nkb in GROUPS for h in range(HPC)]

            diag_sl = {}
            far_sl = {}
            mid_sl = {}

            def emit_qk(t):
                h, kb0, nkb = tasks[t]
                st = stp.tile([TB, GW], fp32, tag="st", name=f"st_{t}")
                # layout: [d0 f0 d1 f1 ...] then [m0 m1 ...], contiguous.
                # skip blocks whose query block is past the sequence end
                # (their pt/e slices are never consumed by PV).
                specs = []
                for i in range(nkb):
                    kb = kb0 + i
                    for dst, src in [
                        (2 * i * TB, kb * TB),                   # diag
                        ((2 * nkb + i) * TB, kb * TB + TB),      # mid
                        ((2 * i + 1) * TB, kb * TB + 2 * TB),    # far
                    ]:
                        if src + TB <= S:
                            specs.append((dst, src, kb))

                bank_last = {}
                for dst, src, kb in specs:
                    bank_last[dst // 512] = dst
                started_banks = set()
                for dst, src, kb in specs:
                    bank = dst // 512
                    nc.tensor.matmul(
                        st[:, dst:dst + TB],
                        lhsT=kt_sb[:, kb * TB:(kb + 1) * TB],
                        rhs=qt_sb[h][:, src:src + TB],
                        start=(bank not in started_banks),
                        stop=(bank_last[bank] == dst))
                    started_banks.add(bank)
                return st

            def emit_exp_mask(t, st):
                h, kb0, nkb = tasks[t]
                w = 3 * nkb * TB
                e = expp.tile([TB, GW], fp16, tag="exp", name=f"e_{t}")
                nc.scalar.activation(
                    e[:, 0:w], st[:, 0:w], mybir.ActivationFunctionType.Exp)
                pt = ptp.tile([TB, 2 * G * TB], fp16, tag="pt", name=f"pt_{t}")
                dfw = 2 * nkb * TB
                e3 = e[:, 0:dfw].rearrange("p (b c) -> p b c", c=2 * TB)
                p3 = pt[:, 0:dfw].rearrange("p (b c) -> p b c", c=2 * TB)
                m3 = mask_sb[:].unsqueeze(1).broadcast_to([TB, nkb, 2 * TB])
                nc.vector.tensor_mul(p3, e3, m3)
                for i in range(nkb):
                    kb = kb0 + i
                    diag_sl[(h, kb)] = pt[:, 2 * i * TB:(2 * i + 1) * TB]
                    far_sl[(h, kb)] = pt[:, (2 * i + 1) * TB:(2 * i + 2) * TB]
                    mid_sl[(h, kb)] = e[:, (2 * nkb + i) * TB:
                                        (2 * nkb + i + 1) * TB]

            def emit_pv_out(t):
                h, kb0, nkb = tasks[t]
                ct = ctxp.tile([TB, G * VW], fp32, tag="ctx", name=f"ctx_{t}")
                first = True
                for j in range(nkb):
                    qb = kb0 + j
                    # mid first: its lhsT (e slice) only needs exp, while
                    # diag/far need the DVE mask -- by the time the mids
                    # stream, the mask has landed, so the PE never stalls.
                    parts = []
                    if qb >= 1:
                        parts.append((mid_sl[(h, qb - 1)], qb - 1))
                    if qb >= 2:
                        parts.append((far_sl[(h, qb - 2)], qb - 2))
                    parts.append((diag_sl[(h, qb)], qb))
                    for sl, kb in parts:
                        last = (j == nkb - 1) and (kb == qb)
                        nc.tensor.matmul(
                            ct[:, j * VW:(j + 1) * VW], lhsT=sl,
                            rhs=va_sb[h][:, kb * VW:(kb + 1) * VW],
                            start=first, stop=last)
                        first = False
                # raw ctx+denominator: PSUM fp32 -> SBUF fp16 (GpSimd can't
                # read PSUM on TRN2); normalization happens host-side.
                ow = nkb * VW
                o = outp.tile([TB, G * VW], fp16, tag="out", name=f"o_{t}")
                nc.vector.tensor_copy(o[:, 0:ow], ct[:, 0:ow])
                c0 = (h * NKB + kb0) * VW
                nc.sync.dma_start(ctx_d.ap()[:, c0:c0 + ow], o[:, 0:ow])

            # paired schedule: QK(2p), QK(2p+1), PV(2p-2), PV(2p-1) --
            # one QK->PV weight-width switch per pair instead of per task
            # (each 64-row <-> 128-row LDWEIGHTS switch drains the PE
            # pipeline for ~240ns).
            st_tiles = {}
            for p in range(NT // 2 + 1):
                for t in (2 * p, 2 * p + 1):
                    if t < NT:
                        st_tiles[t] = emit_qk(t)
                for t in (2 * p - 2, 2 * p - 1):
                    if 0 <= t:
                        emit_pv_out(t)
                for t in (2 * p, 2 * p + 1):
                    if t < NT:
                        emit_exp_mask(t, st_tiles.pop(t))
    nc.finalize()
    return nc


def _build_causal():
    """Correctness fallback for even layer_idx (full causal attention)."""
    import concourse.bass as bass
    import concourse.tile as tile
    from concourse import bacc, mybir

    fp16 = mybir.dt.float16
    fp32 = mybir.dt.float32
    mwidth = 512

    nc = bacc.Bacc("TRN2", target_bir_lowering=False, debug=False)
    qt_d = nc.dram_tensor("qt", [TB, S], fp16, kind="ExternalInput")
    kt_d = nc.dram_tensor("kt", [TB, S], fp16, kind="ExternalInput")
    va_d = nc.dram_tensor("va", [TB, HPC * NKB * VW], fp16, kind="ExternalInput")
    mask_d = nc.dram_tensor("mask", [TB, mwidth], fp16, kind="ExternalInput")
    ctx_d = nc.dram_tensor("ctx", [HPC, S, DK], fp32, kind="ExternalOutput")

    with tile.TileContext(nc) as tc:
        with (
            tc.tile_pool(name="inp", bufs=1) as inp,
            tc.tile_pool(name="exp", bufs=3) as expp,
            tc.tile_pool(name="pt", bufs=4) as ptp,
            tc.tile_pool(name="stp", bufs=2, space="PSUM") as stp,
            tc.tile_pool(name="ctxp", bufs=4, space="PSUM") as ctxp,
            tc.tile_pool(name="outp", bufs=3) as outp,
        ):
            mask_sb = inp.tile([TB, mwidth], fp16, tag="mask")
            nc.sync.dma_start(mask_sb[:], mask_d.ap())
            qt_sb = inp.tile([TB, S], fp16, tag="qt")
            nc.sync.dma_start(qt_sb[:], qt_d.ap())
            kt_sb = inp.tile([TB, S], fp16, tag="kt")
            nc.sync.dma_start(kt_sb[:], kt_d.ap())
            va_sb = inp.tile([TB, HPC * NKB * VW], fp16, tag="va")
            nc.sync.dma_start(va_sb[:], va_d.ap())

            for h in range(HPC):
                hr = slice(h * DK, (h + 1) * DK)
                ctx_tiles = {}
                started = set()
                for kb in range(NKB):
                    span = S - kb * TB
                    chunks = []
                    for o in range(0, span, 512):
                        w = min(512, span - o)
                        st = stp.tile([TB, 512], fp32, tag="st",
                                      name=f"st_{h}_{kb}_{o}")
                        nc.tensor.matmul(
                            st[:, 0:w], lhsT=kt_sb[hr, kb * TB:kb * TB + TB],
                            rhs=qt_sb[hr, kb * TB + o:kb * TB + o + w],
                            start=True, stop=True)
                        pt = ptp.tile([TB, 512], fp16, tag="pt",
                                      name=f"pt_{h}_{kb}_{o}")
                        if o == 0:
                            e = expp.tile([TB, 512], fp16, tag="exp",
                                          name=f"e_{h}_{kb}_{o}")
                            nc.scalar.activation(
                                e[:, 0:w], st[:, 0:w],
                                mybir.ActivationFunctionType.Exp)
                            nc.vector.tensor_mul(
                                pt[:, 0:w], e[:, 0:w], mask_sb[:, 0:w])
                        else:
                            nc.scalar.activation(
                                pt[:, 0:w], st[:, 0:w],
                                mybir.ActivationFunctionType.Exp)
                        chunks.append(pt)

                    for qb in range(kb, NKB):
                        g, j = divmod(qb, G)
                        if g not in ctx_tiles:
                            ctx_tiles[g] = ctxp.tile(
                                [TB, G * VW], fp32, tag="ctx", name=f"ctx_{h}_{g}")
                        ct = ctx_tiles[g]
                        o = (qb - kb) * TB
                        src = chunks[o // 512]
                        oo = o % 512
                        last = (qb == g * G + G - 1) and (kb == qb)
                        nc.tensor.matmul(
                            ct[:, j * VW:(j + 1) * VW],
                            lhsT=src[:, oo:oo + TB],
                            rhs=va_sb[:, (h * NKB + kb) * VW:(h * NKB + kb + 1) * VW],
                            start=(g not in started), stop=last)
                        started.add(g)
                        if last:
                            ct3 = ct[:].rearrange("p (n c) -> p n c", c=VW)
                            recip = outp.tile([TB, G], fp32, tag="recip",
                                              name=f"recip_{h}_{g}")
                            nc.vector.reciprocal(recip[:], ct3[:, :, DK])
                            out_sb = outp.tile([TB, G * DK], fp32, tag="out",
                                               name=f"out_{h}_{g}")
                            out3 = out_sb[:].rearrange("p (n c) -> p n c", c=DK)
                            nc.vector.tensor_mul(
                                out3, ct3[:, :, 0:DK],
                                recip[:].unsqueeze(2).broadcast_to([TB, G, DK]))
                            dst = ctx_d.ap()[h, g * G * TB:(g + 1) * G * TB, :]
                            dst = dst.rearrange("(n p) d -> p n d", p=TB)
                            nc.sync.dma_start(dst, out3)
                            del ctx_tiles[g]
                            started.discard(g)
    nc.finalize()
    return nc


def _get_program(win):
    if win not in _prog_cache:
        _prog_cache[win] = (
            _build_banded() if win == LOCAL_WINDOW else _build_causal())
    return _prog_cache[win]


def _make_mask_np_causal():
    kl = np.arange(TB)[:, None]
    qs = np.arange(512)[None, :]
    return ((qs - kl) >= 0).astype(np.float16)


def make_in_maps(q, k, v, win):
    scale = np.float32(1.0 / np.sqrt(DK))
    in_maps = []
    for c in range(N_CORES):
        heads = range(c * HPC, (c + 1) * HPC)
        qt = np.concatenate(
            [(q[0, h] * scale).T for h in heads], axis=0).astype(np.float16)
        kt = np.concatenate(
            [k[0, h].T for h in heads], axis=0).astype(np.float16)
        va = np.empty((TB, HPC * NKB * VW), np.float16)
        for hi, h in enumerate(heads):
            vh = np.concatenate(
                [v[0, h], np.ones((S, 1), np.float32)], axis=1)  # [S, 65]
            va[:, hi * NKB * VW:(hi + 1) * NKB * VW] = (
                vh.reshape(NKB, TB, VW).transpose(1, 0, 2).reshape(TB, NKB * VW)
            ).astype(np.float16)
        m = {
            "qt": np.ascontiguousarray(qt),
            "kt": np.ascontiguousarray(kt),
            "va": np.ascontiguousarray(va),
        }
        if win != LOCAL_WINDOW:
            m["mask"] = _make_mask_np_causal()
        in_maps.append(m)
    return in_maps


def decode_ctx(out, win):
    """Decode one core's 'ctx' result to [HPC, S, DK] fp32."""
    if win != LOCAL_WINDOW:
        return np.asarray(out, np.float32)
    # banded layout: [TB, HPC*NKB*VW] fp16 raw ctx+denominator; column
    # (h*NKB + qb)*VW + c holds ctx (c<DK) / denom (c=DK) for query
    # qb*TB + p of head h.
    a = np.asarray(out, np.float32).reshape(TB, HPC, NKB, VW)
    o = a[..., 0:DK] / a[..., DK:DK + 1]        # [TB, HPC, NKB, DK]
    o = o.transpose(1, 2, 0, 3)                 # [HPC, NKB, TB, DK]
    return np.ascontiguousarray(o.reshape(HPC, S, DK))


def kernel(q, k, v, layer_idx=1, training=0):
    from concourse.bass_utils import run_bass_kernel_spmd

    q = np.asarray(q)
    k = np.asarray(k)
    v = np.asarray(v)
    li = int(np.asarray(layer_idx))
    win = S if li % 2 == 0 else LOCAL_WINDOW

    nc = _get_program(win)
    in_maps = make_in_maps(q, k, v, win)
    res = run_bass_kernel_spmd(nc, in_maps, core_ids=list(range(N_CORES)))

    ctx = np.empty((B, H, S, DK), np.float32)
    for c in range(N_CORES):
        out = decode_ctx(res.results[c]["ctx"], win)
        for hi in range(HPC):
            ctx[0, c * HPC + hi] = out[hi]
    return ctx, k, v
